# revision 1
# baseline (speedup 1.0000x reference)
"""Trainium2 Bass kernel for nn_Decoder_36953898615460.

recon[B, D] = einsum('lbf,lfd->bd', acts[:n], W[:n]) + sum(bias[:n], 0)

Strategy (row-parallel over F, 8 NeuronCores):
  - Shard the contraction dim F across 8 cores: core r owns F columns
    [r*768, (r+1)*768)  ->  local contraction K_loc = n*768.
  - Host prep (pure layout): acts shard transposed to [K_loc, B] so the
    contraction dim lands on SBUF partitions with contiguous DMA; W shard
    reshaped to [K_loc, D]; bias transposed to [D, n].
  - Per core: partial[D, B] (output transposed: d on partitions) computed
    as fp32r (TF32) matmuls accumulating in PSUM per K-chunk, chunk results
    accumulated into an SBUF fp32 accumulator.
  - B is processed in two halves, each with a full K pass and its own
    ReduceScatter(add); the first RS overlaps the second half's compute,
    so only the second RS is exposed at the tail. (W is streamed twice;
    DMA stays under the PE roofline.)
  - bias: each core adds sum_l(bias)/8 so the 8-way reduce sums to +bias.
  - Device-side ReduceScatter(add): core r ends with rows [r*96, (r+1)*96)
    of the reduced [D, B].
  - Host: concat the 8 shards -> [D, B], transpose -> [B, D].
"""

import numpy as np

import concourse.mybir as mybir
import concourse.tile as tile
from concourse import bacc
from concourse.bass import ts
from concourse.bass_utils import run_bass_kernel_spmd

NCORES = 8
B, F, D = 2048, 6144, 768
F_LOC = F // NCORES  # 768
P = 128
NFREE = 512          # matmul moving free dim (one PSUM bank of fp32)
CK = 6               # k-tiles (of 128) per chunk
HALVES = 2           # B split; each half gets a full K pass + its own RS
# Symmetric split: half-1's 227 us PE window absorbs the W re-stream plus
# RS_0's SDMA traffic with margin (a 1536/512 split starves half-1's DMA:
# measured 618 us vs 588 us symmetric).
BHS = [1024, 1024]
BOFF = [0, 1024]
PARTIAL_DT = mybir.dt.float32  # RS wire format (bf16 would be ~4% faster at ~20x the error; fp32 keeps rel err at the fp32r compute floor 1.5e-4)

_nc_cache = {}
last_result = None  # BassKernelResults of the most recent run (for test harness)


def _build(n_layers: int):
    K_LOC = n_layers * F_LOC          # 9216 for n=12
    KT = K_LOC // P                   # 72 k-tiles
    assert KT % CK == 0
    NCH = KT // CK                    # 12 chunks
    MD = D // P                       # 6 d-subtiles
    DR = D // NCORES                  # 96 rows per rank after ReduceScatter

    nc = bacc.Bacc(None, num_devices=NCORES)
    a_ext = nc.dram_tensor("a_t", [K_LOC, B], mybir.dt.float32r, kind="ExternalInput")
    w_ext = nc.dram_tensor("w", [K_LOC, D], mybir.dt.float32r, kind="ExternalInput")
    b_ext = nc.dram_tensor("bias_t", [D, n_layers], mybir.dt.float32, kind="ExternalInput")
    y_ext = nc.dram_tensor("y", [DR, B], PARTIAL_DT, kind="ExternalOutput")

    partials = [
        nc.dram_tensor(f"partial{h}", [D, BHS[h]], PARTIAL_DT) for h in range(HALVES)
    ]
    reduceds = [
        nc.dram_tensor(f"reduced{h}", [DR, BHS[h]], PARTIAL_DT) for h in range(HALVES)
    ]

    a_v = a_ext[:, :].rearrange("(ko p) b -> p ko b", p=P)  # [128, KT, B]
    w_v = w_ext[:, :].rearrange("(ko p) d -> p ko d", p=P)  # [128, KT, D]
    b_v = b_ext[:, :].rearrange("(mo p) l -> p mo l", p=P)  # [128, MD, n]

    with tile.TileContext(nc) as tc:
        with (
            tc.tile_pool(name="apool", bufs=2) as apool,
            tc.tile_pool(name="wpool", bufs=2) as wpool,
            tc.tile_pool(name="cpool", bufs=1) as cpool,
            tc.tile_pool(name="opool", bufs=2) as opool,
            tc.tile_pool(name="pspool", bufs=3, space="PSUM") as pspool,
        ):
            # bias8[p, mo] = sum_l bias[l, mo*128+p] / NCORES
            bias_t = cpool.tile([P, MD, n_layers], mybir.dt.float32)
            nc.sync.dma_start(bias_t[:], b_v)
            bias8 = cpool.tile([P, MD], mybir.dt.float32)
            nc.vector.reduce_sum(bias8[:], bias_t[:], axis=mybir.AxisListType.X)
            nc.vector.tensor_scalar_mul(bias8[:], bias8[:], 1.0 / NCORES)

            for h in range(HALVES):
                b0, BH = BOFF[h], BHS[h]
                NB = BH // NFREE
                # fp32 accumulator for this half's partial, acc[p, mo, b]
                acc = cpool.tile([P, MD, BH], mybir.dt.float32, tag="acc")
                for c in range(NCH):
                    a_c = apool.tile([P, CK, BH], mybir.dt.float32r, tag="a")
                    w_c = wpool.tile([P, CK, D], mybir.dt.float32r, tag="w")
                    for k in range(CK):
                        nc.sync.dma_start(
                            a_c[:, k], a_v[:, c * CK + k, b0 : b0 + BH]
                        )
                        nc.sync.dma_start(w_c[:, k], w_v[:, c * CK + k])
                    for m in range(MD):
                        ps = pspool.tile([P, BH], mybir.dt.float32, tag="ps")
                        for k in range(CK):
                            lhsT = w_c[:, k, ts(m, P)]
                            for nb in range(NB):
                                nc.tensor.matmul(
                                    ps[:, ts(nb, NFREE)],
                                    lhsT,
                                    a_c[:, k, ts(nb, NFREE)],
                                    start=(k == 0),
                                    stop=(k == CK - 1),
                                )
                        if c == 0:
                            nc.vector.tensor_scalar_add(
                                acc[:, m], ps[:], bias8[:, m : m + 1]
                            )
                        else:
                            nc.vector.tensor_add(acc[:, m], ps[:], acc[:, m])

                # write this half's partial (convert only if wire dtype differs)
                for m in range(MD):
                    if PARTIAL_DT == mybir.dt.float32:
                        nc.sync.dma_start(partials[h][ts(m, P), :], acc[:, m])
                    else:
                        pb = opool.tile([P, BH], PARTIAL_DT, tag="pb")
                        nc.vector.tensor_copy(pb[:], acc[:, m])
                        nc.sync.dma_start(partials[h][ts(m, P), :], pb[:])

                nc.gpsimd.collective_compute(
                    "ReduceScatter",
                    mybir.AluOpType.add,
                    replica_groups=[list(range(NCORES))],
                    ins=[partials[h][:, :].opt()],
                    outs=[reduceds[h][:, :].opt()],
                )

            # Final output DMAs last, on the SWDGE (gpsimd) queue: a y-DMA
            # waits on its RS completion, and a waiting DMA at the head of
            # the sync HWDGE queue would stall the second half's input
            # streaming behind it (measured 41 us PE gap).
            for h in range(HALVES):
                nc.gpsimd.dma_start(
                    y_ext[:, BOFF[h] : BOFF[h] + BHS[h]], reduceds[h][:, :]
                )
    nc.compile()
    return nc


def _get_nc(n_layers: int):
    if n_layers not in _nc_cache:
        _nc_cache[n_layers] = _build(n_layers)
    return _nc_cache[n_layers]


def kernel(acts: np.ndarray, W: np.ndarray, bias: np.ndarray, layer_idx) -> np.ndarray:
    global last_result
    n = int(layer_idx) + 1
    acts = np.asarray(acts, dtype=np.float32)[:n]  # [n, B, F]
    W = np.asarray(W, dtype=np.float32)[:n]        # [n, F, D]
    bias = np.asarray(bias, dtype=np.float32)[:n]  # [n, D]

    nc = _get_nc(n)

    bias_t = np.ascontiguousarray(bias.T)  # [D, n], same on every core
    in_maps = []
    for r in range(NCORES):
        f0 = r * F_LOC
        # [n, B, F_LOC] -> [n, F_LOC, B] -> [K_loc, B]
        a_t = np.ascontiguousarray(acts[:, :, f0 : f0 + F_LOC].transpose(0, 2, 1)).reshape(
            n * F_LOC, B
        )
        w_r = np.ascontiguousarray(W[:, f0 : f0 + F_LOC, :]).reshape(n * F_LOC, D)
        in_maps.append({"a_t": a_t, "w": w_r, "bias_t": bias_t})

    last_result = run_bass_kernel_spmd(nc, in_maps, core_ids=list(range(NCORES)))
    out_t = np.concatenate([last_result.results[r]["y"] for r in range(NCORES)], axis=0)
    return np.ascontiguousarray(out_t.T.astype(np.float32))  # [B, D] float32



# revision 2
# speedup vs baseline: 1.2961x; 1.2961x over previous
"""Trainium2 Bass kernel for nn_Decoder_36953898615460.

recon[B, D] = einsum('lbf,lfd->bd', acts[:n], W[:n]) + sum(bias[:n], 0)

Strategy (2-way F x 4-way B sharding, 8 NeuronCores):
  - Cores form 4 pairs; pair p owns B block [p*512, (p+1)*512). Within a
    pair, core r%2 owns F half [r%2 * 3072, ...) -> local contraction
    K_loc = n*3072 (288 k-tiles at n=12).
  - Rationale vs the baseline's 8-way F split: the 8-rank ReduceScatter
    (mesh algo) costs ~63-71 us and is exposed at the tail; a 2-rank RS
    over a quarter of the data is far cheaper. The price is that W is
    streamed at 2x total (each F-half read by 4 pairs), but with fp16
    inputs total DMA is 94 MB/core = ~264 us @ 358 GB/s, well under the
    ~373 us PE floor.
  - fp16 inputs (host-side cast, free wrt HW time): halves DMA traffic
    vs fp32/fp32r and enables FWL fast weight load; matmul rate is
    1 cycle/row for fp16 and fp32r alike, so no PE-time change.
  - Single full-K pass: 6 PSUM banks (one per 128-row D subtile) hold
    the [768, 512] partial accumulated across all 24n k-tiles
    (start at k-tile 0, stop at the last) -- no intermediate SBUF
    accumulator, no per-chunk vector adds.
  - Host prep is partition-major: a2[p, ko, b], w2[p, ko, d] so each
    chunk DMA is one contiguous multi-KB read per partition.
  - bias: each core of a pair adds sum_l(bias)/2 during the PSUM drain
    so the 2-way reduce sums to +bias.
  - Tail: drain PSUM (+bias) -> fp16 partial [768, 512] -> 2-rank
    ReduceScatter(add) within the pair -> y [384, 512] fp16.
  - Host: assemble 8x [384, 512] -> [768, 2048] -> transpose -> fp32.
"""

import numpy as np

import concourse.mybir as mybir
import concourse.tile as tile
from concourse import bacc
from concourse.bass import ts
from concourse.bass_utils import run_bass_kernel_spmd

NCORES = 8
B, F, D = 2048, 6144, 768
FSPLIT = 2
BSPLIT = 4
F_LOC = F // FSPLIT   # 3072
B_LOC = B // BSPLIT   # 512
P = 128
MD = D // P           # 6 d-subtiles
DR = D // FSPLIT      # 384 rows per core after the pair ReduceScatter
CK = 8                # k-tiles (of 128) per DMA chunk
IN_DT = mybir.dt.float16
WIRE_DT = mybir.dt.float16

_nc_cache = {}
last_result = None  # BassKernelResults of the most recent run (for test harness)


def _build(n_layers: int):
    K_LOC = n_layers * F_LOC          # 36864 for n=12
    KT = K_LOC // P                   # 288 k-tiles
    assert KT % CK == 0
    NCH = KT // CK                    # 36 chunks

    nc = bacc.Bacc(None, num_devices=NCORES)
    a_ext = nc.dram_tensor("a2", [P, KT, B_LOC], IN_DT, kind="ExternalInput")
    w_ext = nc.dram_tensor("w2", [P, KT, D], IN_DT, kind="ExternalInput")
    b_ext = nc.dram_tensor("bias_t", [D, n_layers], mybir.dt.float32, kind="ExternalInput")
    y_ext = nc.dram_tensor("y", [DR, B_LOC], WIRE_DT, kind="ExternalOutput")

    partial = nc.dram_tensor("partial", [D, B_LOC], WIRE_DT)
    reduced = nc.dram_tensor("reduced", [DR, B_LOC], WIRE_DT)

    b_v = b_ext[:, :].rearrange("(mo p) l -> p mo l", p=P)        # [128, 6, n]
    partial_v = partial[:, :].rearrange("(mo p) b -> p mo b", p=P)  # [128, 6, 512]

    with tile.TileContext(nc) as tc:
        with (
            tc.tile_pool(name="apool", bufs=3) as apool,
            tc.tile_pool(name="wpool", bufs=3) as wpool,
            tc.tile_pool(name="cpool", bufs=1) as cpool,
            tc.tile_pool(name="opool", bufs=1) as opool,
            tc.tile_pool(name="pspool", bufs=1, space="PSUM") as pspool,
        ):
            # Persistent accumulator: 6 PSUM banks, bank m = D rows
            # [m*128, (m+1)*128) x B_LOC columns.
            ps = pspool.tile([P, MD, B_LOC], mybir.dt.float32)

            # Prefetch the first chunks before the (vector-only) bias prep
            # so input streaming owns the head of the sync DMA queue.
            chunks = []
            for c in range(min(2, NCH)):
                a_c = apool.tile([P, CK, B_LOC], IN_DT, tag="a")
                w_c = wpool.tile([P, CK, D], IN_DT, tag="w")
                nc.sync.dma_start(a_c[:], a_ext[:, c * CK : (c + 1) * CK, :])
                nc.sync.dma_start(w_c[:], w_ext[:, c * CK : (c + 1) * CK, :])
                chunks.append((a_c, w_c))

            # bias2[p, mo] = sum_l bias[l, mo*128+p] / FSPLIT
            bias_t = cpool.tile([P, MD, n_layers], mybir.dt.float32)
            nc.sync.dma_start(bias_t[:], b_v)
            bias2 = cpool.tile([P, MD], mybir.dt.float32)
            nc.vector.reduce_sum(bias2[:], bias_t[:], axis=mybir.AxisListType.X)
            nc.vector.tensor_scalar_mul(bias2[:], bias2[:], 1.0 / FSPLIT)

            for c in range(NCH):
                if c < len(chunks):
                    a_c, w_c = chunks[c]
                else:
                    a_c = apool.tile([P, CK, B_LOC], IN_DT, tag="a")
                    w_c = wpool.tile([P, CK, D], IN_DT, tag="w")
                    nc.sync.dma_start(a_c[:], a_ext[:, c * CK : (c + 1) * CK, :])
                    nc.sync.dma_start(w_c[:], w_ext[:, c * CK : (c + 1) * CK, :])
                for k in range(CK):
                    first = c == 0 and k == 0
                    last = c == NCH - 1 and k == CK - 1
                    for m in range(MD):
                        nc.tensor.matmul(
                            ps[:, m],
                            w_c[:, k, ts(m, P)],
                            a_c[:, k],
                            start=first,
                            stop=last,
                        )

            # Drain: partial[mo*128+p, b] = ps + bias/2, in wire dtype.
            out_t = opool.tile([P, MD, B_LOC], WIRE_DT)
            for m in range(MD):
                nc.vector.tensor_scalar_add(out_t[:, m], ps[:, m], bias2[:, m : m + 1])
            nc.sync.dma_start(partial_v, out_t[:])

            nc.gpsimd.collective_compute(
                "ReduceScatter",
                mybir.AluOpType.add,
                replica_groups=[[2 * p, 2 * p + 1] for p in range(BSPLIT)],
                ins=[partial[:, :].opt()],
                outs=[reduced[:, :].opt()],
            )
            nc.gpsimd.dma_start(y_ext[:, :], reduced[:, :])
    nc.compile()
    return nc


def _get_nc(n_layers: int):
    if n_layers not in _nc_cache:
        _nc_cache[n_layers] = _build(n_layers)
    return _nc_cache[n_layers]


def kernel(acts: np.ndarray, W: np.ndarray, bias: np.ndarray, layer_idx) -> np.ndarray:
    global last_result
    n = int(layer_idx) + 1
    acts = np.asarray(acts, dtype=np.float32)[:n]  # [n, B, F]
    W = np.asarray(W, dtype=np.float32)[:n]        # [n, F, D]
    bias = np.asarray(bias, dtype=np.float32)[:n]  # [n, D]

    nc = _get_nc(n)

    KT = n * F_LOC // P
    FO = F_LOC // P  # 24 f-subtiles per core
    acts16 = acts.astype(np.float16)
    W16 = W.astype(np.float16)
    bias_t = np.ascontiguousarray(bias.T)  # [D, n] fp32, same on every core

    in_maps = []
    for r in range(NCORES):
        pair, fh = r // 2, r % 2
        b0, f0 = pair * B_LOC, fh * F_LOC
        # a2[p, (l, fo), b] = acts[l, b0+b, f0 + fo*128 + p]
        a2 = np.ascontiguousarray(
            acts16[:, b0 : b0 + B_LOC, f0 : f0 + F_LOC]
            .reshape(n, B_LOC, FO, P)
            .transpose(3, 0, 2, 1)
            .reshape(P, KT, B_LOC)
        )
        # w2[p, (l, fo), d] = W[l, f0 + fo*128 + p, d]
        w2 = np.ascontiguousarray(
            W16[:, f0 : f0 + F_LOC, :]
            .reshape(n, FO, P, D)
            .transpose(2, 0, 1, 3)
            .reshape(P, KT, D)
        )
        in_maps.append({"a2": a2, "w2": w2, "bias_t": bias_t})

    last_result = run_bass_kernel_spmd(nc, in_maps, core_ids=list(range(NCORES)))
    # Core 2p has D rows [0, 384), core 2p+1 rows [384, 768) of B block p.
    full = np.empty((D, B), dtype=np.float16)
    for r in range(NCORES):
        pair, fh = r // 2, r % 2
        full[fh * DR : (fh + 1) * DR, pair * B_LOC : (pair + 1) * B_LOC] = (
            last_result.results[r]["y"]
        )
    return full.T.astype(np.float32)  # [B, D] float32


# revision 3
# speedup vs baseline: 1.3937x; 1.0753x over previous
"""Trainium2 Bass kernel for nn_Decoder_36953898615460.

recon[B, D] = einsum('lbf,lfd->bd', acts[:n], W[:n]) + sum(bias[:n], 0)

Strategy (2-way F x 4-way B sharding, 8 NeuronCores):
  - Cores form 4 pairs; pair p owns B block [p*512, (p+1)*512). Within a
    pair, core r%2 owns F half [r%2 * 3072, ...) -> local contraction
    K_loc = n*3072 (288 k-tiles at n=12).
  - Rationale vs the baseline's 8-way F split: the 8-rank ReduceScatter
    (mesh algo) costs ~63-71 us and is exposed at the tail; a 2-rank RS
    over a quarter of the data is far cheaper. The price is that W is
    streamed at 2x total (each F-half read by 4 pairs), but with fp16
    inputs total DMA is 94 MB/core = ~264 us @ 358 GB/s, well under the
    ~373 us PE floor.
  - fp16 inputs (host-side cast, free wrt HW time): halves DMA traffic
    vs fp32/fp32r and enables FWL fast weight load; matmul rate is
    1 cycle/row for fp16 and fp32r alike, so no PE-time change.
  - Single full-K pass: 6 PSUM banks (one per 128-row D subtile) hold
    the [768, 512] partial accumulated across all 24n k-tiles
    (start at k-tile 0, stop at the last) -- no intermediate SBUF
    accumulator, no per-chunk vector adds.
  - Host prep is partition-major: a2[p, ko, b], w2[p, ko, d] so each
    chunk DMA is one contiguous multi-KB read per partition.
  - bias: each core of a pair adds sum_l(bias)/2 during the PSUM drain
    so the 2-way reduce sums to +bias.
  - Tail: drain PSUM (+bias) -> fp16 partial [768, 512] -> 2-rank
    ReduceScatter(add) within the pair -> y [384, 512] fp16.
  - Host: assemble 8x [384, 512] -> [768, 2048] -> transpose -> fp32.
"""

import numpy as np

import concourse.mybir as mybir
import concourse.tile as tile
from concourse import bacc
from concourse.bass import ts
from concourse.bass_utils import run_bass_kernel_spmd

NCORES = 8
B, F, D = 2048, 6144, 768
FSPLIT = 2
BSPLIT = 4
F_LOC = F // FSPLIT   # 3072
B_LOC = B // BSPLIT   # 512
P = 128
MD = D // P           # 6 d-subtiles
DR = D // FSPLIT      # 384 rows per core after the pair ReduceScatter
CK = 8                # k-tiles (of 128) per DMA chunk
IN_DT = mybir.dt.float16
WIRE_DT = mybir.dt.float16

_nc_cache = {}
last_result = None  # BassKernelResults of the most recent run (for test harness)


def _build(n_layers: int):
    K_LOC = n_layers * F_LOC          # 36864 for n=12
    KT = K_LOC // P                   # 288 k-tiles
    assert KT % CK == 0
    NCH = KT // CK                    # 36 chunks

    # Chunk schedule: small first chunks so the PE starts ~10 us earlier,
    # then steady CK-sized chunks. Sums to KT for any n >= 1.
    chunk_sizes = [2, 2, 4] + [CK] * (KT // CK - 1)
    assert sum(chunk_sizes) == KT
    PAIRS = [[2 * p, 2 * p + 1] for p in range(BSPLIT)]

    nc = bacc.Bacc(None, num_devices=NCORES)
    a_ext = nc.dram_tensor("a2", [P, KT, B_LOC], IN_DT, kind="ExternalInput")
    w_ext = nc.dram_tensor("w2", [P, KT, D], IN_DT, kind="ExternalInput")
    b_ext = nc.dram_tensor("bias_t", [D, n_layers], mybir.dt.float32, kind="ExternalInput")
    y_ext = nc.dram_tensor("y", [DR, B_LOC], WIRE_DT, kind="ExternalOutput")

    partial = nc.dram_tensor("partial", [D, B_LOC], WIRE_DT)
    reduced = nc.dram_tensor("reduced", [DR, B_LOC], WIRE_DT)
    # Tiny scratch for a warm-up collective: the first collective in a NEFF
    # pays ~11 us of ncfw wake-up latency (measured); later ones ~1 us.
    warm_in = nc.dram_tensor("warm_in", [2, 16], mybir.dt.float32)
    warm_out = nc.dram_tensor("warm_out", [1, 16], mybir.dt.float32)

    b_v = b_ext[:, :].rearrange("(mo p) l -> p mo l", p=P)        # [128, 6, n]
    partial_v = partial[:, :].rearrange("(mo p) b -> p mo b", p=P)  # [128, 6, 512]

    with tile.TileContext(nc) as tc:
        with (
            tc.tile_pool(name="apool", bufs=3) as apool,
            tc.tile_pool(name="wpool", bufs=3) as wpool,
            tc.tile_pool(name="cpool", bufs=1) as cpool,
            tc.tile_pool(name="opool", bufs=1) as opool,
            tc.tile_pool(name="pspool", bufs=1, space="PSUM") as pspool,
        ):
            # Persistent accumulator: 6 PSUM banks, bank m = D rows
            # [m*128, (m+1)*128) x B_LOC columns.
            ps = pspool.tile([P, MD, B_LOC], mybir.dt.float32)

            # Prefetch the first chunks before the (vector-only) bias prep
            # so input streaming owns the head of the sync DMA queue.
            chunks = []
            k0 = 0
            for ck in chunk_sizes[:3]:
                a_c = apool.tile([P, ck, B_LOC], IN_DT, tag=f"a{ck}")
                w_c = wpool.tile([P, ck, D], IN_DT, tag=f"w{ck}")
                nc.sync.dma_start(a_c[:], a_ext[:, k0 : k0 + ck, :])
                nc.sync.dma_start(w_c[:], w_ext[:, k0 : k0 + ck, :])
                chunks.append((a_c, w_c))
                k0 += ck

            # Warm up the collective firmware while the PE streams.
            nc.gpsimd.collective_compute(
                "ReduceScatter",
                mybir.AluOpType.add,
                replica_groups=PAIRS,
                ins=[warm_in[:, :].opt()],
                outs=[warm_out[:, :].opt()],
            )

            # bias2[p, mo] = sum_l bias[l, mo*128+p] / FSPLIT
            bias_t = cpool.tile([P, MD, n_layers], mybir.dt.float32)
            nc.sync.dma_start(bias_t[:], b_v)
            bias2 = cpool.tile([P, MD], mybir.dt.float32)
            nc.vector.reduce_sum(bias2[:], bias_t[:], axis=mybir.AxisListType.X)
            nc.vector.tensor_scalar_mul(bias2[:], bias2[:], 1.0 / FSPLIT)

            out_t = opool.tile([P, MD, B_LOC], WIRE_DT)
            NCH = len(chunk_sizes)
            k0 = 0
            for c, ck in enumerate(chunk_sizes):
                if c < len(chunks):
                    a_c, w_c = chunks[c]
                else:
                    a_c = apool.tile([P, ck, B_LOC], IN_DT, tag=f"a{ck}")
                    w_c = wpool.tile([P, ck, D], IN_DT, tag=f"w{ck}")
                    nc.sync.dma_start(a_c[:], a_ext[:, k0 : k0 + ck, :])
                    nc.sync.dma_start(w_c[:], w_ext[:, k0 : k0 + ck, :])
                if c < NCH - 1:
                    for k in range(ck):
                        first = c == 0 and k == 0
                        for m in range(MD):
                            nc.tensor.matmul(
                                ps[:, m],
                                w_c[:, k, ts(m, P)],
                                a_c[:, k],
                                start=first,
                                stop=False,
                            )
                else:
                    # Final chunk: m-outer so subtile m finishes (and drains,
                    # and writes its partial slice) while m+1.. still stream.
                    for m in range(MD):
                        for k in range(ck):
                            nc.tensor.matmul(
                                ps[:, m],
                                w_c[:, k, ts(m, P)],
                                a_c[:, k],
                                start=False,
                                stop=k == ck - 1,
                            )
                        nc.vector.tensor_scalar_add(
                            out_t[:, m], ps[:, m], bias2[:, m : m + 1]
                        )
                        nc.sync.dma_start(partial_v[:, m], out_t[:, m])
                k0 += ck

            nc.gpsimd.collective_compute(
                "ReduceScatter",
                mybir.AluOpType.add,
                replica_groups=PAIRS,
                ins=[partial[:, :].opt()],
                outs=[reduced[:, :].opt()],
            )
            nc.gpsimd.dma_start(y_ext[:, :], reduced[:, :])
    nc.compile()
    return nc


def _get_nc(n_layers: int):
    if n_layers not in _nc_cache:
        _nc_cache[n_layers] = _build(n_layers)
    return _nc_cache[n_layers]


def kernel(acts: np.ndarray, W: np.ndarray, bias: np.ndarray, layer_idx) -> np.ndarray:
    global last_result
    n = int(layer_idx) + 1
    acts = np.asarray(acts, dtype=np.float32)[:n]  # [n, B, F]
    W = np.asarray(W, dtype=np.float32)[:n]        # [n, F, D]
    bias = np.asarray(bias, dtype=np.float32)[:n]  # [n, D]

    nc = _get_nc(n)

    KT = n * F_LOC // P
    FO = F_LOC // P  # 24 f-subtiles per core
    acts16 = acts.astype(np.float16)
    W16 = W.astype(np.float16)
    bias_t = np.ascontiguousarray(bias.T)  # [D, n] fp32, same on every core

    in_maps = []
    for r in range(NCORES):
        pair, fh = r // 2, r % 2
        b0, f0 = pair * B_LOC, fh * F_LOC
        # a2[p, (l, fo), b] = acts[l, b0+b, f0 + fo*128 + p]
        a2 = np.ascontiguousarray(
            acts16[:, b0 : b0 + B_LOC, f0 : f0 + F_LOC]
            .reshape(n, B_LOC, FO, P)
            .transpose(3, 0, 2, 1)
            .reshape(P, KT, B_LOC)
        )
        # w2[p, (l, fo), d] = W[l, f0 + fo*128 + p, d]
        w2 = np.ascontiguousarray(
            W16[:, f0 : f0 + F_LOC, :]
            .reshape(n, FO, P, D)
            .transpose(2, 0, 1, 3)
            .reshape(P, KT, D)
        )
        in_maps.append({"a2": a2, "w2": w2, "bias_t": bias_t})

    last_result = run_bass_kernel_spmd(nc, in_maps, core_ids=list(range(NCORES)))
    # Core 2p has D rows [0, 384), core 2p+1 rows [384, 768) of B block p.
    full = np.empty((D, B), dtype=np.float16)
    for r in range(NCORES):
        pair, fh = r // 2, r % 2
        full[fh * DR : (fh + 1) * DR, pair * B_LOC : (pair + 1) * B_LOC] = (
            last_result.results[r]["y"]
        )
    return full.T.astype(np.float32)  # [B, D] float32
